# revision 15
# baseline (speedup 1.0000x reference)
"""Trainium2 Bass kernel for a 3-layer GNN message-passing network.

Sharding: data-parallel over batch B=8, one batch element per NeuronCore.
Layout strategy: feature-major activations feed every GEMM (in-channels on
partitions), token-major copies are used for LayerNorm; PE transposes switch
orientation.  Degree normalizations (div, div_e) are folded into
host-precomputed G/div and (G/div_e)^T arrays so the kernel never divides by
the degrees on-chip.

Precision: matmul operands are bf16 (PE runs 4x faster than fp32 and DMA
traffic halves); all accumulation (PSUM), residual state, LayerNorm and
softmax denominators stay fp32.  Set KERNEL_MMDT=f32 for an exact-fp32 build.
"""

import os
from contextlib import ExitStack

import ml_dtypes
import numpy as np

import concourse.bass as bass
import concourse.mybir as mybir
import concourse.tile as tile
from concourse import bacc
from concourse.bass_utils import run_bass_kernel_spmd
from concourse.masks import make_identity

FP = mybir.dt.float32
BF = mybir.dt.bfloat16

# Matmul operand dtype knob.
MD = FP if os.environ.get("KERNEL_MMDT", "bf16") == "f32" else BF
MD_NP = np.float32 if MD == FP else ml_dtypes.bfloat16

# Problem constants (hardcoded per contract; kernel.py must be self-contained)
B = 8
N = 1024          # nodes
E = 4096          # edges
C = 256           # node/edge feature dim
H = 8             # heads
HD = 32           # head dim
HID = 1024        # FFN hidden dims (node and both edge FFNs)
NL = 3            # layers
SCALE = 1.0 / np.sqrt(HD)
EPS = 1e-5
P = 128
NB = N // P       # 8 node blocks
EB = E // P       # 32 edge blocks
ECH = 512         # edge chunk size for the edge phase
NCH = E // ECH    # 16 chunks
CB = C // P       # 2 channel blocks

LAST_RESULTS = None  # BassKernelResults of the most recent run (for test.py)


def _mm(nc, out, lhsT, rhs, start, stop, tile_position=None):
    nc.tensor.matmul(out, lhsT, rhs, start=start, stop=stop,
                     tile_position=tile_position)


def _bcast_ap(dram_ap, parts):
    # [D] dram vector -> [parts, D] AP with partition step 0 (DMA broadcast)
    return bass.AP(
        tensor=dram_ap.tensor,
        offset=dram_ap.offset,
        ap=[[0, parts]] + [list(d) for d in dram_ap.ap],
    )


def _ln_tok(nc, pool, x, gB, bB, eps_t):
    """LayerNorm along the free axis of token-major x [128, C], in place.
    gB/bB are [128, C] partition-broadcast gamma/beta tiles."""
    stats = pool.tile([P, nc.vector.BN_STATS_DIM], FP, tag="ln_stats")
    nc.vector.bn_stats(out=stats, in_=x)
    mv = pool.tile([P, nc.vector.BN_AGGR_DIM], FP, tag="ln_mv")
    nc.vector.bn_aggr(out=mv, in_=stats)
    rstd = pool.tile([P, 1], FP, tag="ln_rstd")
    nc.scalar.activation(out=rstd, in_=mv[:, 1:2],
                         func=mybir.ActivationFunctionType.Sqrt, bias=eps_t)
    nc.vector.reciprocal(out=rstd, in_=rstd)
    nc.vector.tensor_scalar(out=x, in0=x, scalar1=mv[:, 0:1], scalar2=rstd,
                            op0=mybir.AluOpType.subtract,
                            op1=mybir.AluOpType.mult)
    nc.vector.tensor_mul(out=x, in0=x, in1=gB)
    nc.vector.tensor_add(out=x, in0=x, in1=bB)


def build_kernel():
    nc = bacc.Bacc("TRN2", target_bir_lowering=False, debug=False)

    # ---- DRAM I/O (per-core shapes) ----
    nf_in = nc.dram_tensor("nf_in", (N, C), FP, kind="ExternalInput")
    nfT_in = nc.dram_tensor("nfT_in", (C, N), MD, kind="ExternalInput")
    ef_in = nc.dram_tensor("ef_in", (E, C), FP, kind="ExternalInput")
    ef16_in = nc.dram_tensor("ef16_in", (E, C), MD, kind="ExternalInput")
    gdiv = nc.dram_tensor("gdiv", (E, N), MD, kind="ExternalInput")   # G/div
    gts = nc.dram_tensor("gts", (N, E), MD, kind="ExternalInput")  # (G/dive)^T
    ind = nc.dram_tensor("ind", (H, C), MD, kind="ExternalInput")  # head 1-hot
    W = []
    for i in range(NL):
        d = {}
        def t(name, shape, dt, i=i, d=d):
            d[name] = nc.dram_tensor(f"L{i}_{name}", shape, dt,
                                     kind="ExternalInput")
        t("wqk", (2 * C, 2 * C), MD); t("bqk", (2 * C,), FP)
        t("wv", (2 * C, C), MD); t("bv", (C,), FP)
        t("wo", (C, C), MD); t("bo", (C,), FP)
        t("wn1", (C, HID), MD); t("bn1", (HID,), FP)
        t("wn2", (HID, C), MD); t("bn2", (C,), FP)
        t("we1a", (2 * C, HID), MD); t("be1a", (HID,), FP)
        t("we1b", (HID, C), MD); t("be1b", (C,), FP)
        t("we2a", (C, HID), MD); t("be2a", (HID,), FP)
        t("we2b", (HID, C), MD); t("be2b", (C,), FP)
        for nm in ("g1n", "b1n", "g2n", "b2n", "g1e", "b1e", "g2e", "b2e"):
            t(nm, (C,), FP)
        W.append(d)
    nf_out = nc.dram_tensor("nf_out", (N, C), FP, kind="ExternalOutput")

    gdiv_r = gdiv[:].rearrange("(eo p) n -> p eo n", p=P)      # [128, 32, 1024]
    gts_r = gts[:].rearrange("(no p) e -> p no e", p=P)        # [128, 8, 4096]

    with tile.TileContext(nc) as tc, ExitStack() as ctx:
        const = ctx.enter_context(tc.tile_pool(name="const", bufs=1))
        persist = ctx.enter_context(tc.tile_pool(name="persist", bufs=1))
        wp = ctx.enter_context(tc.tile_pool(name="wp", bufs=1))
        bp = ctx.enter_context(tc.tile_pool(name="bp", bufs=1))
        scr = ctx.enter_context(tc.tile_pool(name="scr", bufs=2))
        small = ctx.enter_context(tc.tile_pool(name="small", bufs=3))
        # PSUM budget: 8 banks total = ps(4) + ps_acc(2 x bufs=1) + ps_tr(2)
        ps = ctx.enter_context(tc.tile_pool(name="ps", bufs=4, space="PSUM"))
        ps_acc = ctx.enter_context(
            tc.tile_pool(name="ps_acc", bufs=1, space="PSUM"))
        ps_tr = ctx.enter_context(
            tc.tile_pool(name="ps_tr", bufs=2, space="PSUM"))

        ident = const.tile([P, P], FP)
        make_identity(nc, ident)
        eps_t = const.tile([P, 1], FP)
        nc.vector.memset(eps_t, EPS)
        ind_sb = const.tile([H, C], MD)
        nc.sync.dma_start(ind_sb, ind[:])

        # Persistent state (fp32) + bf16 matmul-operand copies
        nf_tok = persist.tile([P, NB, C], FP, tag="nf_tok")
        ef_tok = persist.tile([P, EB, C], FP, tag="ef_tok")
        nf16 = persist.tile([P, NB, C], MD, tag="nf16")
        ef16 = persist.tile([P, EB, C], MD, tag="ef16")
        XT = persist.tile([P, 4, N], MD, tag="XT")  # [0:2]=nfT, [2:4]=naggT
        v_aug = persist.tile([P, NB, H, HD + 1], MD, tag="v_aug")

        nc.sync.dma_start(nf_tok, nf_in[:].rearrange("(o p) c -> p o c", p=P))
        nc.sync.dma_start(ef_tok, ef_in[:].rearrange("(o p) c -> p o c", p=P))
        nc.sync.dma_start(ef16, ef16_in[:].rearrange("(o p) c -> p o c", p=P))
        nc.sync.dma_start(XT[:, 0:2, :],
                          nfT_in[:].rearrange("(o p) n -> p o n", p=P))
        nc.vector.memset(v_aug[:, :, :, HD:HD + 1], 1.0)

        def transpose_128(src_ap, dst_ap):
            """PE-transpose a fp32 [128,128] sbuf block into dst via psum."""
            pt = ps_tr.tile([P, P], FP, tag="tr")
            nc.tensor.transpose(pt, src_ap, ident)
            nc.vector.tensor_copy(out=dst_ap, in_=pt)

        def load_w(dram, kdim, mdim, tag):
            w = wp.tile([P, kdim // P, mdim], MD, tag=tag)
            nc.sync.dma_start(w, dram[:].rearrange("(ko p) m -> p ko m", p=P))
            return w

        def load_b(dram, dim, tag):
            b = bp.tile([P, dim // P], FP, tag=tag)
            nc.sync.dma_start(b, dram[:].rearrange("(o p) -> p o", p=P))
            return b

        def load_bcast(dram, dim, tag):
            b = bp.tile([P, dim], FP, tag=tag)
            nc.gpsimd.dma_start(out=b, in_=_bcast_ap(dram[:], P))
            return b

        for li in range(NL):
            w = W[li]

            # ---------------- node phase ----------------
            # naggT[c, n] = sum_e ef[e, c] * gdiv[e, n]  -> XT[2:4]
            for nh in range(2):  # n free halves of 512
                pss = [ps_acc.tile([P, 512], FP, tag=f"acc{cb}", bufs=1,
                                   name=f"nagg_ps{cb}")
                       for cb in range(CB)]
                for eb in range(EB):
                    gt = small.tile([P, 512], MD, tag="gdiv_t", bufs=2)
                    nc.sync.dma_start(gt,
                                      gdiv_r[:, eb, nh * 512:(nh + 1) * 512])
                    for cb in range(CB):
                        _mm(nc, pss[cb], ef16[:, eb, cb * P:(cb + 1) * P],
                            gt, start=(eb == 0), stop=(eb == EB - 1))
                for cb in range(CB):
                    nc.vector.tensor_copy(
                        out=XT[:, 2 + cb, nh * 512:(nh + 1) * 512], in_=pss[cb])

            # qkT [512, 1024] feature-major
            wqk = load_w(w["wqk"], 2 * C, 2 * C, "w_a")
            bqk = load_b(w["bqk"], 2 * C, "b_a")
            qkT = scr.tile([P, 4, N], MD, tag="big4k", bufs=1)
            for cb in range(4):
                for nh in range(2):
                    pt = ps.tile([P, 512], FP, tag="mm")
                    for kb in range(4):
                        _mm(nc, pt, wqk[:, kb, cb * P:(cb + 1) * P],
                            XT[:, kb, nh * 512:(nh + 1) * 512],
                            start=(kb == 0), stop=(kb == 3))
                    nc.vector.tensor_scalar_add(
                        out=qkT[:, cb, nh * 512:(nh + 1) * 512], in0=pt,
                        scalar1=bqk[:, cb:cb + 1])

            # v token-major [1024, 256] into v_aug[..., 0:32]
            wv = load_w(w["wv"], 2 * C, C, "w_b")
            bvB = load_bcast(w["bv"], C, "bB")
            for nb in range(NB):
                pt = ps.tile([P, C], FP, tag="mm")
                for kb in range(4):
                    _mm(nc, pt, XT[:, kb, nb * P:(nb + 1) * P], wv[:, kb, :],
                        start=(kb == 0), stop=(kb == 3))
                nc.vector.tensor_add(
                    out=v_aug[:, nb, :, 0:HD],
                    in0=pt.rearrange("p (h c) -> p h c", h=H),
                    in1=bvB.rearrange("p (h c) -> p h c", h=H))

            # attention
            attn_catT = scr.tile([P, CB, N], MD, tag="cat16")
            denom = scr.tile([H, N], FP, tag="denom", bufs=1)
            denom16 = scr.tile([H, N], MD, tag="denom16", bufs=1)
            for h in range(H):
                qT_h = qkT[HD * (h % 4):HD * (h % 4) + HD, h // 4, :]
                kT_h = qkT[HD * (h % 4):HD * (h % 4) + HD, 2 + h // 4, :]
                pnum = [ps_acc.tile([HD + 1, 512], FP, tag=f"acc{nh}",
                                    bufs=1, name=f"num_ps{nh}")
                        for nh in range(2)]
                for mb in range(NB):
                    et = small.tile([P, N], MD, tag="E", bufs=3)
                    for nh in range(2):
                        pst = ps.tile([P, 512], FP, tag="mm")
                        _mm(nc, pst, kT_h[:, mb * P:(mb + 1) * P],
                            qT_h[:, nh * 512:(nh + 1) * 512],
                            start=True, stop=True,
                            tile_position=(HD * (h % 4), 0))
                        nc.scalar.activation(
                            out=et[:, nh * 512:(nh + 1) * 512], in_=pst,
                            func=mybir.ActivationFunctionType.Exp, scale=SCALE)
                    for nh in range(2):
                        _mm(nc, pnum[nh], v_aug[:, mb, h, :],
                            et[:, nh * 512:(nh + 1) * 512],
                            start=(mb == 0), stop=(mb == NB - 1))
                dstage = small.tile([1, N], FP, tag="dstage", bufs=1)
                for nh in range(2):
                    nc.vector.tensor_copy(
                        out=attn_catT[HD * (h % 4):HD * (h % 4) + HD, h // 4,
                                      nh * 512:(nh + 1) * 512],
                        in_=pnum[nh][0:HD, :])
                    nc.scalar.activation(
                        out=dstage[:, nh * 512:(nh + 1) * 512],
                        in_=pnum[nh][HD:HD + 1, :],
                        func=mybir.ActivationFunctionType.Copy)
                nc.sync.dma_start(denom[h:h + 1, :], dstage)

            # normalize: attn_catT *= bcast(1/denom)
            nc.vector.reciprocal(out=denom, in_=denom)
            nc.vector.tensor_copy(out=denom16, in_=denom)
            for cb in range(CB):
                for nh in range(2):
                    pb = ps.tile([P, 512], FP, tag="mm")
                    _mm(nc, pb, ind_sb[:, cb * P:(cb + 1) * P],
                        denom16[:, nh * 512:(nh + 1) * 512],
                        start=True, stop=True)
                    nc.vector.tensor_mul(
                        out=attn_catT[:, cb, nh * 512:(nh + 1) * 512],
                        in0=attn_catT[:, cb, nh * 512:(nh + 1) * 512], in1=pb)

            # new_nodesT = Wo^T @ attn_catT + bo (fp32, feeds transposes only)
            wo = load_w(w["wo"], C, C, "w_a")
            bo = load_b(w["bo"], C, "b_a")
            nnT = scr.tile([P, CB, N], FP, tag="cat1024")
            for cb in range(CB):
                for nh in range(2):
                    pt = ps.tile([P, 512], FP, tag="mm")
                    for kb in range(CB):
                        _mm(nc, pt, wo[:, kb, cb * P:(cb + 1) * P],
                            attn_catT[:, kb, nh * 512:(nh + 1) * 512],
                            start=(kb == 0), stop=(kb == CB - 1))
                    nc.vector.tensor_scalar_add(
                        out=nnT[:, cb, nh * 512:(nh + 1) * 512], in0=pt,
                        scalar1=bo[:, cb:cb + 1])

            # residual + LN1 (token-major)
            g1nB = load_bcast(w["g1n"], C, "gB")
            b1nB = load_bcast(w["b1n"], C, "bB2")
            for nb in range(NB):
                for cb in range(CB):
                    pt = ps_tr.tile([P, P], FP, tag="tr")
                    nc.tensor.transpose(pt, nnT[:, cb, nb * P:(nb + 1) * P],
                                        ident)
                    nc.vector.tensor_add(
                        out=nf_tok[:, nb, cb * P:(cb + 1) * P],
                        in0=nf_tok[:, nb, cb * P:(cb + 1) * P], in1=pt)
                _ln_tok(nc, small, nf_tok[:, nb, :], g1nB, b1nB, eps_t)

            # nfT (into XT[0:2]) for FFN rhs
            for nb in range(NB):
                for cb in range(CB):
                    transpose_128(nf_tok[:, nb, cb * P:(cb + 1) * P],
                                  XT[:, cb, nb * P:(nb + 1) * P])

            # node FFN: ff = relu(nf@Wn1+b)@Wn2+b, two hidden halves
            wn1 = load_w(w["wn1"], C, HID, "w_a")
            bn1 = load_b(w["bn1"], HID, "b_hid")
            wn2 = load_w(w["wn2"], HID, C, "w_b")
            bn2 = load_b(w["bn2"], C, "b_a")
            h1T = scr.tile([P, NB, N], MD, tag="big8k", bufs=1)
            for hb in range(NB):
                for nh in range(2):
                    pt = ps.tile([P, 512], FP, tag="mm")
                    for kb in range(CB):
                        _mm(nc, pt, wn1[:, kb, hb * P:(hb + 1) * P],
                            XT[:, kb, nh * 512:(nh + 1) * 512],
                            start=(kb == 0), stop=(kb == CB - 1))
                    nc.scalar.activation(
                        out=h1T[:, hb, nh * 512:(nh + 1) * 512], in_=pt,
                        func=mybir.ActivationFunctionType.Relu,
                        bias=bn1[:, hb:hb + 1])
            ffT = scr.tile([P, CB, N], FP, tag="cat1024")
            for cb in range(CB):
                for nh in range(2):
                    pff = ps_acc.tile([P, 512], FP, tag=f"acc{nh}", bufs=1)
                    for kb in range(NB):
                        _mm(nc, pff,
                            wn2[:, kb, cb * P:(cb + 1) * P],
                            h1T[:, kb, nh * 512:(nh + 1) * 512],
                            start=(kb == 0), stop=(kb == NB - 1))
                    nc.vector.tensor_scalar_add(
                        out=ffT[:, cb, nh * 512:(nh + 1) * 512],
                        in0=pff,
                        scalar1=bn2[:, cb:cb + 1])

            # residual + LN2; rebuild nfT (XT[0:2]) and nf16
            g2nB = load_bcast(w["g2n"], C, "gB")
            b2nB = load_bcast(w["b2n"], C, "bB2")
            for nb in range(NB):
                for cb in range(CB):
                    pt = ps_tr.tile([P, P], FP, tag="tr")
                    nc.tensor.transpose(pt, ffT[:, cb, nb * P:(nb + 1) * P],
                                        ident)
                    nc.vector.tensor_add(
                        out=nf_tok[:, nb, cb * P:(cb + 1) * P],
                        in0=nf_tok[:, nb, cb * P:(cb + 1) * P], in1=pt)
                _ln_tok(nc, small, nf_tok[:, nb, :], g2nB, b2nB, eps_t)
                nc.vector.tensor_copy(out=nf16[:, nb, :], in_=nf_tok[:, nb, :])
            for nb in range(NB):
                for cb in range(CB):
                    transpose_128(nf_tok[:, nb, cb * P:(cb + 1) * P],
                                  XT[:, cb, nb * P:(nb + 1) * P])

            # ---------------- edge phase ----------------
            we1a = load_w(w["we1a"], 2 * C, HID, "w_a")
            be1a = load_b(w["be1a"], HID, "b_hid")
            we1b = load_w(w["we1b"], HID, C, "w_b")
            be1b = load_b(w["be1b"], C, "b_a")
            we2a = load_w(w["we2a"], C, HID, "w_c")
            be2a = load_b(w["be2a"], HID, "b_hid2")
            we2b = load_w(w["we2b"], HID, C, "w_d")
            be2b = load_b(w["be2b"], C, "b_b")
            g1eB = load_bcast(w["g1e"], C, "gB")
            b1eB = load_bcast(w["b1e"], C, "bB2")
            g2eB = load_bcast(w["g2e"], C, "gB2")
            b2eB = load_bcast(w["b2e"], C, "bB3")

            for ec in range(NCH):
                esl = slice(ec * ECH, (ec + 1) * ECH)
                # eaggT [2cb, 256e] = nf^T(G/div_e)^T  (feature-major)
                gtt = scr.tile([P, NB, ECH], MD, tag="gts_t", bufs=2)
                nc.sync.dma_start(gtt, gts_r[:, :, esl])
                eaggT = scr.tile([P, CB, ECH], MD, tag="eaggT", bufs=2)
                for cb in range(CB):
                    pt = ps.tile([P, ECH], FP, tag="mm")
                    for kb in range(NB):
                        _mm(nc, pt, nf16[:, kb, cb * P:(cb + 1) * P],
                            gtt[:, kb, :], start=(kb == 0),
                            stop=(kb == NB - 1))
                    nc.vector.tensor_copy(out=eaggT[:, cb, :], in_=pt)
                # efT chunk (feature-major, pre-LN ef)
                efTc = scr.tile([P, CB, ECH], MD, tag="efTc", bufs=2)
                for eb in range(ECH // P):
                    for cb in range(CB):
                        transpose_128(
                            ef_tok[:, ec * (ECH // P) + eb, cb * P:(cb + 1) * P],
                            efTc[:, cb, eb * P:(eb + 1) * P])
                # h1 = relu([ef, eagg] @ We1a + b)
                h1e = scr.tile([P, HID // P, ECH], MD, tag="h1e", bufs=1)
                for hb in range(HID // P):
                    pt = ps.tile([P, ECH], FP, tag="mm")
                    for kb in range(CB):
                        _mm(nc, pt, we1a[:, kb, hb * P:(hb + 1) * P],
                            efTc[:, kb, :], start=(kb == 0), stop=False)
                    for kb in range(CB):
                        _mm(nc, pt, we1a[:, CB + kb, hb * P:(hb + 1) * P],
                            eaggT[:, kb, :], start=False, stop=(kb == CB - 1))
                    nc.vector.tensor_scalar(
                        out=h1e[:, hb, :], in0=pt,
                        scalar1=be1a[:, hb:hb + 1], scalar2=0.0,
                        op0=mybir.AluOpType.add, op1=mybir.AluOpType.max)
                # e1T = We1b^T @ h1 + b   (fp32, feeds transpose/residual)
                e1T = scr.tile([P, CB, ECH], FP, tag="e1T", bufs=2)
                for cb in range(CB):
                    pt = ps.tile([P, ECH], FP, tag="mm")
                    for kb in range(HID // P):
                        _mm(nc, pt, we1b[:, kb, cb * P:(cb + 1) * P],
                            h1e[:, kb, :], start=(kb == 0),
                            stop=(kb == HID // P - 1))
                    nc.vector.tensor_scalar_add(
                        out=e1T[:, cb, :], in0=pt,
                        scalar1=be1b[:, cb:cb + 1])
                # residual + LN1 (token-major)
                for eb in range(ECH // P):
                    for cb in range(CB):
                        pt = ps_tr.tile([P, P], FP, tag="tr")
                        nc.tensor.transpose(pt,
                                            e1T[:, cb, eb * P:(eb + 1) * P],
                                            ident)
                        nc.vector.tensor_add(
                            out=ef_tok[:, ec * (ECH // P) + eb,
                                       cb * P:(cb + 1) * P],
                            in0=ef_tok[:, ec * (ECH // P) + eb,
                                       cb * P:(cb + 1) * P],
                            in1=pt)
                    _ln_tok(nc, small, ef_tok[:, ec * (ECH // P) + eb, :],
                            g1eB, b1eB, eps_t)
                # efT of LN1 output
                for eb in range(ECH // P):
                    for cb in range(CB):
                        transpose_128(
                            ef_tok[:, ec * (ECH // P) + eb, cb * P:(cb + 1) * P],
                            efTc[:, cb, eb * P:(eb + 1) * P])
                # h2 = relu(ef @ We2a + b)
                for hb in range(HID // P):
                    pt = ps.tile([P, ECH], FP, tag="mm")
                    for kb in range(CB):
                        _mm(nc, pt, we2a[:, kb, hb * P:(hb + 1) * P],
                            efTc[:, kb, :], start=(kb == 0),
                            stop=(kb == CB - 1))
                    nc.vector.tensor_scalar(
                        out=h1e[:, hb, :], in0=pt,
                        scalar1=be2a[:, hb:hb + 1], scalar2=0.0,
                        op0=mybir.AluOpType.add, op1=mybir.AluOpType.max)
                # e2T = We2b^T @ h2 + b
                for cb in range(CB):
                    pt = ps.tile([P, ECH], FP, tag="mm")
                    for kb in range(HID // P):
                        _mm(nc, pt, we2b[:, kb, cb * P:(cb + 1) * P],
                            h1e[:, kb, :], start=(kb == 0),
                            stop=(kb == HID // P - 1))
                    nc.vector.tensor_scalar_add(
                        out=e1T[:, cb, :], in0=pt,
                        scalar1=be2b[:, cb:cb + 1])
                # residual + LN2; refresh ef16
                for eb in range(ECH // P):
                    for cb in range(CB):
                        pt = ps_tr.tile([P, P], FP, tag="tr")
                        nc.tensor.transpose(pt,
                                            e1T[:, cb, eb * P:(eb + 1) * P],
                                            ident)
                        nc.vector.tensor_add(
                            out=ef_tok[:, ec * (ECH // P) + eb,
                                       cb * P:(cb + 1) * P],
                            in0=ef_tok[:, ec * (ECH // P) + eb,
                                       cb * P:(cb + 1) * P],
                            in1=pt)
                    _ln_tok(nc, small, ef_tok[:, ec * (ECH // P) + eb, :],
                            g2eB, b2eB, eps_t)
                    if li < NL - 1:
                        nc.vector.tensor_copy(
                            out=ef16[:, ec * (ECH // P) + eb, :],
                            in_=ef_tok[:, ec * (ECH // P) + eb, :])

        # output
        nc.sync.dma_start(nf_out[:].rearrange("(o p) c -> p o c", p=P), nf_tok)

    nc.compile()
    return nc


def _prep_host(node_features, edge_features, G, params):
    nf = np.ascontiguousarray(np.asarray(node_features, np.float32))
    ef = np.ascontiguousarray(np.asarray(edge_features, np.float32))
    Gf = np.asarray(G, np.float32)
    div = Gf.sum(axis=1)                      # [B, N]
    div_e = Gf.sum(axis=2)                    # [B, E]
    gdiv = np.ascontiguousarray((Gf / div[:, None, :]).astype(MD_NP))
    gts = np.ascontiguousarray(
        (Gf / div_e[:, :, None]).transpose(0, 2, 1).astype(MD_NP))
    nfT = np.ascontiguousarray(nf.transpose(0, 2, 1).astype(MD_NP))
    ef16 = ef.astype(MD_NP)

    ind = np.zeros((H, C), np.float32)
    for h in range(H):
        ind[h, HD * h:HD * (h + 1)] = 1.0
    ind = ind.astype(MD_NP)

    layers = []
    for p in params:
        p = {k: np.asarray(v, np.float32) for k, v in p.items()}
        Wqn = p["Wqn"].reshape(C, H, 3, HD)
        Wqe = p["Wqe"].reshape(C, H, 3, HD)
        bqn = p["bqn"].reshape(H, 3, HD)
        bqe = p["bqe"].reshape(H, 3, HD)
        bsum = bqn + bqe
        wq = np.concatenate([Wqn[:, :, 0].reshape(C, C),
                             Wqe[:, :, 0].reshape(C, C)], axis=0)
        wk = np.concatenate([Wqn[:, :, 1].reshape(C, C),
                             Wqe[:, :, 1].reshape(C, C)], axis=0)
        wv = np.concatenate([Wqn[:, :, 2].reshape(C, C),
                             Wqe[:, :, 2].reshape(C, C)], axis=0)
        d = {
            "wqk": np.concatenate([wq, wk], axis=1).astype(MD_NP),
            "bqk": np.concatenate([bsum[:, 0].reshape(C),
                                   bsum[:, 1].reshape(C)]),
            "wv": wv.astype(MD_NP),
            "bv": bsum[:, 2].reshape(C),
            "wo": p["Wo"].astype(MD_NP), "bo": p["bo"],
            "wn1": p["Wn1"].astype(MD_NP), "bn1": p["bn1"],
            "wn2": p["Wn2"].astype(MD_NP), "bn2": p["bn2"],
            "we1a": p["We1a"].astype(MD_NP), "be1a": p["be1a"],
            "we1b": p["We1b"].astype(MD_NP), "be1b": p["be1b"],
            "we2a": p["We2a"].astype(MD_NP), "be2a": p["be2a"],
            "we2b": p["We2b"].astype(MD_NP), "be2b": p["be2b"],
            "g1n": p["g1n"], "b1n": p["b1n"], "g2n": p["g2n"], "b2n": p["b2n"],
            "g1e": p["g1e"], "b1e": p["b1e"], "g2e": p["g2e"], "b2e": p["b2e"],
        }
        layers.append({k: np.ascontiguousarray(v) for k, v in d.items()})
    return nf, nfT, ef, ef16, gdiv, gts, ind, layers


_NC_CACHE = None


def kernel(node_features, edge_features, G, params):
    global LAST_RESULTS, _NC_CACHE
    nf, nfT, ef, ef16, gdiv, gts, ind, layers = _prep_host(
        node_features, edge_features, G, params)

    if _NC_CACHE is None:
        _NC_CACHE = build_kernel()
    nc = _NC_CACHE

    shared = {"ind": ind}
    for i, d in enumerate(layers):
        for k, v in d.items():
            shared[f"L{i}_{k}"] = v
    in_maps = []
    for b in range(B):
        m = dict(shared)
        m["nf_in"] = nf[b]
        m["nfT_in"] = nfT[b]
        m["ef_in"] = ef[b]
        m["ef16_in"] = ef16[b]
        m["gdiv"] = gdiv[b]
        m["gts"] = gts[b]
        in_maps.append(m)

    res = run_bass_kernel_spmd(nc, in_maps, core_ids=list(range(B)))
    LAST_RESULTS = res
    out = np.stack([res.results[b]["nf_out"] for b in range(B)], axis=0)
    return (out, np.asarray(G))


# revision 21
# speedup vs baseline: 1.0680x; 1.0680x over previous
"""Trainium2 Bass kernel for a 3-layer GNN message-passing network.

Sharding: data-parallel over batch B=8, one batch element per NeuronCore.
Layout strategy: feature-major activations feed every GEMM (in-channels on
partitions), token-major copies are used for LayerNorm; PE transposes switch
orientation.  Degree normalizations (div, div_e) are folded into
host-precomputed G/div and (G/div_e)^T arrays so the kernel never divides by
the degrees on-chip.

Precision: matmul operands are bf16 (PE runs 4x faster than fp32 and DMA
traffic halves); all accumulation (PSUM), residual state, LayerNorm and
softmax denominators stay fp32.  Set KERNEL_MMDT=f32 for an exact-fp32 build.
"""

import os
from contextlib import ExitStack

import ml_dtypes
import numpy as np

import concourse.bass as bass
import concourse.mybir as mybir
import concourse.tile as tile
from concourse import bacc
from concourse.bass_utils import run_bass_kernel_spmd
from concourse.masks import make_identity

FP = mybir.dt.float32
BF = mybir.dt.bfloat16

# Matmul operand dtype knob.
MD = FP if os.environ.get("KERNEL_MMDT", "bf16") == "f32" else BF
MD_NP = np.float32 if MD == FP else ml_dtypes.bfloat16

# Problem constants (hardcoded per contract; kernel.py must be self-contained)
B = 8
N = 1024          # nodes
E = 4096          # edges
C = 256           # node/edge feature dim
H = 8             # heads
HD = 32           # head dim
HID = 1024        # FFN hidden dims (node and both edge FFNs)
NL = 3            # layers
SCALE = 1.0 / np.sqrt(HD)
EPS = 1e-5
P = 128
NB = N // P       # 8 node blocks
EB = E // P       # 32 edge blocks
ECH = 512         # edge chunk size for the edge phase
NCH = E // ECH    # 16 chunks
CB = C // P       # 2 channel blocks

LAST_RESULTS = None  # BassKernelResults of the most recent run (for test.py)


def _mm(nc, out, lhsT, rhs, start, stop, tile_position=None):
    nc.tensor.matmul(out, lhsT, rhs, start=start, stop=stop,
                     tile_position=tile_position)


def _bcast_ap(dram_ap, parts):
    # [D] dram vector -> [parts, D] AP with partition step 0 (DMA broadcast)
    return bass.AP(
        tensor=dram_ap.tensor,
        offset=dram_ap.offset,
        ap=[[0, parts]] + [list(d) for d in dram_ap.ap],
    )


def _ln_tok(nc, pool, x, gB, bB, eps_t):
    """LayerNorm along the free axis of token-major x [128, C], in place.
    gB/bB are [128, C] partition-broadcast gamma/beta tiles."""
    stats = pool.tile([P, nc.vector.BN_STATS_DIM], FP, tag="ln_stats")
    nc.vector.bn_stats(out=stats, in_=x)
    mv = pool.tile([P, nc.vector.BN_AGGR_DIM], FP, tag="ln_mv")
    nc.vector.bn_aggr(out=mv, in_=stats)
    rstd = pool.tile([P, 1], FP, tag="ln_rstd")
    nc.scalar.activation(out=rstd, in_=mv[:, 1:2],
                         func=mybir.ActivationFunctionType.Sqrt, bias=eps_t)
    nc.vector.reciprocal(out=rstd, in_=rstd)
    nc.vector.tensor_scalar(out=x, in0=x, scalar1=mv[:, 0:1], scalar2=rstd,
                            op0=mybir.AluOpType.subtract,
                            op1=mybir.AluOpType.mult)
    nc.vector.tensor_mul(out=x, in0=x, in1=gB)
    nc.vector.tensor_add(out=x, in0=x, in1=bB)


def build_kernel():
    nc = bacc.Bacc("TRN2", target_bir_lowering=False, debug=False)

    # ---- DRAM I/O (per-core shapes) ----
    nf_in = nc.dram_tensor("nf_in", (N, C), FP, kind="ExternalInput")
    nfT_in = nc.dram_tensor("nfT_in", (C, N), MD, kind="ExternalInput")
    ef_in = nc.dram_tensor("ef_in", (E, C), FP, kind="ExternalInput")
    ef16_in = nc.dram_tensor("ef16_in", (E, C), MD, kind="ExternalInput")
    gdiv = nc.dram_tensor("gdiv", (E, N), MD, kind="ExternalInput")   # G/div
    gts = nc.dram_tensor("gts", (N, E), MD, kind="ExternalInput")  # (G/dive)^T
    ind = nc.dram_tensor("ind", (H, C), MD, kind="ExternalInput")  # head 1-hot
    W = []
    for i in range(NL):
        d = {}
        def t(name, shape, dt, i=i, d=d):
            d[name] = nc.dram_tensor(f"L{i}_{name}", shape, dt,
                                     kind="ExternalInput")
        t("wqk", (2 * C, 2 * C), MD); t("bqk", (2 * C,), FP)
        t("wv", (2 * C, C), MD); t("bv", (C,), FP)
        t("wo", (C, C), MD); t("bo", (C,), FP)
        t("wn1", (C, HID), MD); t("bn1", (HID,), FP)
        t("wn2", (HID, C), MD); t("bn2", (C,), FP)
        t("we1a", (2 * C, HID), MD); t("be1a", (HID,), FP)
        t("we1b", (HID, C), MD); t("be1b", (C,), FP)
        t("we2a", (C, HID), MD); t("be2a", (HID,), FP)
        t("we2b", (HID, C), MD); t("be2b", (C,), FP)
        for nm in ("g1n", "b1n", "g2n", "b2n", "g1e", "b1e", "g2e", "b2e"):
            t(nm, (C,), FP)
        W.append(d)
    nf_out = nc.dram_tensor("nf_out", (N, C), FP, kind="ExternalOutput")

    gdiv_r = gdiv[:].rearrange("(eo p) n -> p eo n", p=P)      # [128, 32, 1024]
    gts_r = gts[:].rearrange("(no p) e -> p no e", p=P)        # [128, 8, 4096]

    with tile.TileContext(nc) as tc, ExitStack() as ctx:
        const = ctx.enter_context(tc.tile_pool(name="const", bufs=1))
        persist = ctx.enter_context(tc.tile_pool(name="persist", bufs=1))
        wp = ctx.enter_context(tc.tile_pool(name="wp", bufs=1))
        bp = ctx.enter_context(tc.tile_pool(name="bp", bufs=1))
        scr = ctx.enter_context(tc.tile_pool(name="scr", bufs=2))
        small = ctx.enter_context(tc.tile_pool(name="small", bufs=3))
        # PSUM budget: 8 banks total = ps(4) + ps_acc(2 x bufs=1) + ps_tr(2)
        ps = ctx.enter_context(tc.tile_pool(name="ps", bufs=4, space="PSUM"))
        ps_acc = ctx.enter_context(
            tc.tile_pool(name="ps_acc", bufs=1, space="PSUM"))
        ps_tr = ctx.enter_context(
            tc.tile_pool(name="ps_tr", bufs=2, space="PSUM"))

        ident = const.tile([P, P], FP)
        make_identity(nc, ident)
        eps_t = const.tile([P, 1], FP)
        nc.vector.memset(eps_t, EPS)
        ind_sb = const.tile([H, C], MD)
        nc.sync.dma_start(ind_sb, ind[:])

        # Persistent state (fp32) + bf16 matmul-operand copies
        nf_tok = persist.tile([P, NB, C], FP, tag="nf_tok")
        ef_tok = persist.tile([P, EB, C], FP, tag="ef_tok")
        nf16 = persist.tile([P, NB, C], MD, tag="nf16")
        ef16 = persist.tile([P, EB, C], MD, tag="ef16")
        XT = persist.tile([P, 4, N], MD, tag="XT")  # [0:2]=nfT, [2:4]=naggT
        v_aug = persist.tile([P, NB, H, HD + 1], MD, tag="v_aug")

        nc.sync.dma_start(nf_tok, nf_in[:].rearrange("(o p) c -> p o c", p=P))
        nc.sync.dma_start(ef_tok, ef_in[:].rearrange("(o p) c -> p o c", p=P))
        nc.sync.dma_start(ef16, ef16_in[:].rearrange("(o p) c -> p o c", p=P))
        nc.sync.dma_start(XT[:, 0:2, :],
                          nfT_in[:].rearrange("(o p) n -> p o n", p=P))
        nc.vector.memset(v_aug[:, :, :, HD:HD + 1], 1.0)

        def transpose_128(src_ap, dst_ap):
            """PE-transpose a fp32 [128,128] sbuf block into dst via psum."""
            pt = ps_tr.tile([P, P], FP, tag="tr")
            nc.tensor.transpose(pt, src_ap, ident)
            nc.scalar.activation(out=dst_ap, in_=pt,
                                 func=mybir.ActivationFunctionType.Copy)

        def load_w(dram, kdim, mdim, tag):
            w = wp.tile([P, kdim // P, mdim], MD, tag=tag)
            nc.sync.dma_start(w, dram[:].rearrange("(ko p) m -> p ko m", p=P))
            return w

        def load_b(dram, dim, tag):
            b = bp.tile([P, dim // P], FP, tag=tag)
            nc.sync.dma_start(b, dram[:].rearrange("(o p) -> p o", p=P))
            return b

        def load_bcast(dram, dim, tag):
            b = bp.tile([P, dim], FP, tag=tag)
            nc.gpsimd.dma_start(out=b, in_=_bcast_ap(dram[:], P))
            return b

        for li in range(NL):
            w = W[li]

            # ---------------- node phase ----------------
            # naggT[c, n] = sum_e ef[e, c] * gdiv[e, n]  -> XT[2:4]
            for nh in range(2):  # n free halves of 512
                pss = [ps_acc.tile([P, 512], FP, tag=f"acc{cb}", bufs=1,
                                   name=f"nagg_ps{cb}")
                       for cb in range(CB)]
                for eb in range(EB):
                    gt = small.tile([P, 512], MD, tag="gdiv_t", bufs=2)
                    nc.sync.dma_start(gt,
                                      gdiv_r[:, eb, nh * 512:(nh + 1) * 512])
                    for cb in range(CB):
                        _mm(nc, pss[cb], ef16[:, eb, cb * P:(cb + 1) * P],
                            gt, start=(eb == 0), stop=(eb == EB - 1))
                for cb in range(CB):
                    nc.scalar.activation(
                        out=XT[:, 2 + cb, nh * 512:(nh + 1) * 512],
                        in_=pss[cb], func=mybir.ActivationFunctionType.Copy)

            # qkT [512, 1024] feature-major
            wqk = load_w(w["wqk"], 2 * C, 2 * C, "w_a")
            bqk = load_b(w["bqk"], 2 * C, "b_a")
            qkT = scr.tile([P, 4, N], MD, tag="big4k", bufs=1)
            for cb in range(4):
                for nh in range(2):
                    pt = ps.tile([P, 512], FP, tag="mm")
                    for kb in range(4):
                        _mm(nc, pt, wqk[:, kb, cb * P:(cb + 1) * P],
                            XT[:, kb, nh * 512:(nh + 1) * 512],
                            start=(kb == 0), stop=(kb == 3))
                    nc.vector.tensor_scalar_add(
                        out=qkT[:, cb, nh * 512:(nh + 1) * 512], in0=pt,
                        scalar1=bqk[:, cb:cb + 1])

            # v token-major [1024, 256] into v_aug[..., 0:32]
            wv = load_w(w["wv"], 2 * C, C, "w_b")
            bvB = load_bcast(w["bv"], C, "bB")
            for nb in range(NB):
                pt = ps.tile([P, C], FP, tag="mm")
                for kb in range(4):
                    _mm(nc, pt, XT[:, kb, nb * P:(nb + 1) * P], wv[:, kb, :],
                        start=(kb == 0), stop=(kb == 3))
                nc.vector.tensor_add(
                    out=v_aug[:, nb, :, 0:HD],
                    in0=pt.rearrange("p (h c) -> p h c", h=H),
                    in1=bvB.rearrange("p (h c) -> p h c", h=H))

            # attention
            attn_catT = scr.tile([P, CB, N], MD, tag="cat16")
            denom = scr.tile([H, N], FP, tag="denom", bufs=1)
            denom16 = scr.tile([H, N], MD, tag="denom16", bufs=1)
            for h in range(H):
                qT_h = qkT[HD * (h % 4):HD * (h % 4) + HD, h // 4, :]
                kT_h = qkT[HD * (h % 4):HD * (h % 4) + HD, 2 + h // 4, :]
                pnum = [ps_acc.tile([HD + 1, 512], FP, tag=f"acc{nh}",
                                    bufs=1, name=f"num_ps{nh}")
                        for nh in range(2)]
                for mb in range(NB):
                    et = small.tile([P, N], MD, tag="E", bufs=3)
                    for nh in range(2):
                        pst = ps.tile([P, 512], FP, tag="mm")
                        _mm(nc, pst, kT_h[:, mb * P:(mb + 1) * P],
                            qT_h[:, nh * 512:(nh + 1) * 512],
                            start=True, stop=True,
                            tile_position=(HD * (h % 4), 0))
                        nc.scalar.activation(
                            out=et[:, nh * 512:(nh + 1) * 512], in_=pst,
                            func=mybir.ActivationFunctionType.Exp, scale=SCALE)
                    for nh in range(2):
                        _mm(nc, pnum[nh], v_aug[:, mb, h, :],
                            et[:, nh * 512:(nh + 1) * 512],
                            start=(mb == 0), stop=(mb == NB - 1))
                dstage = small.tile([1, N], FP, tag="dstage", bufs=1)
                for nh in range(2):
                    nc.vector.tensor_copy(
                        out=attn_catT[HD * (h % 4):HD * (h % 4) + HD, h // 4,
                                      nh * 512:(nh + 1) * 512],
                        in_=pnum[nh][0:HD, :])
                    nc.scalar.activation(
                        out=dstage[:, nh * 512:(nh + 1) * 512],
                        in_=pnum[nh][HD:HD + 1, :],
                        func=mybir.ActivationFunctionType.Copy)
                nc.sync.dma_start(denom[h:h + 1, :], dstage)

            # normalize: attn_catT *= bcast(1/denom)
            nc.vector.reciprocal(out=denom, in_=denom)
            nc.vector.tensor_copy(out=denom16, in_=denom)
            for cb in range(CB):
                for nh in range(2):
                    pb = ps.tile([P, 512], FP, tag="mm")
                    _mm(nc, pb, ind_sb[:, cb * P:(cb + 1) * P],
                        denom16[:, nh * 512:(nh + 1) * 512],
                        start=True, stop=True)
                    nc.vector.tensor_mul(
                        out=attn_catT[:, cb, nh * 512:(nh + 1) * 512],
                        in0=attn_catT[:, cb, nh * 512:(nh + 1) * 512], in1=pb)

            # new_nodesT = Wo^T @ attn_catT + bo (fp32, feeds transposes only)
            wo = load_w(w["wo"], C, C, "w_a")
            bo = load_b(w["bo"], C, "b_a")
            nnT = scr.tile([P, CB, N], FP, tag="cat1024")
            for cb in range(CB):
                for nh in range(2):
                    pt = ps.tile([P, 512], FP, tag="mm")
                    for kb in range(CB):
                        _mm(nc, pt, wo[:, kb, cb * P:(cb + 1) * P],
                            attn_catT[:, kb, nh * 512:(nh + 1) * 512],
                            start=(kb == 0), stop=(kb == CB - 1))
                    nc.vector.tensor_scalar_add(
                        out=nnT[:, cb, nh * 512:(nh + 1) * 512], in0=pt,
                        scalar1=bo[:, cb:cb + 1])

            # residual + LN1 (token-major)
            g1nB = load_bcast(w["g1n"], C, "gB")
            b1nB = load_bcast(w["b1n"], C, "bB2")
            for nb in range(NB):
                for cb in range(CB):
                    pt = ps_tr.tile([P, P], FP, tag="tr")
                    nc.tensor.transpose(pt, nnT[:, cb, nb * P:(nb + 1) * P],
                                        ident)
                    nc.vector.tensor_add(
                        out=nf_tok[:, nb, cb * P:(cb + 1) * P],
                        in0=nf_tok[:, nb, cb * P:(cb + 1) * P], in1=pt)
                _ln_tok(nc, small, nf_tok[:, nb, :], g1nB, b1nB, eps_t)

            # nfT (into XT[0:2]) for FFN rhs
            for nb in range(NB):
                for cb in range(CB):
                    transpose_128(nf_tok[:, nb, cb * P:(cb + 1) * P],
                                  XT[:, cb, nb * P:(nb + 1) * P])

            # node FFN: ff = relu(nf@Wn1+b)@Wn2+b, two hidden halves
            wn1 = load_w(w["wn1"], C, HID, "w_a")
            bn1 = load_b(w["bn1"], HID, "b_hid")
            wn2 = load_w(w["wn2"], HID, C, "w_b")
            bn2 = load_b(w["bn2"], C, "b_a")
            h1T = scr.tile([P, NB, N], MD, tag="big8k", bufs=1)
            for hb in range(NB):
                for nh in range(2):
                    pt = ps.tile([P, 512], FP, tag="mm")
                    for kb in range(CB):
                        _mm(nc, pt, wn1[:, kb, hb * P:(hb + 1) * P],
                            XT[:, kb, nh * 512:(nh + 1) * 512],
                            start=(kb == 0), stop=(kb == CB - 1))
                    nc.scalar.activation(
                        out=h1T[:, hb, nh * 512:(nh + 1) * 512], in_=pt,
                        func=mybir.ActivationFunctionType.Relu,
                        bias=bn1[:, hb:hb + 1])
            ffT = scr.tile([P, CB, N], FP, tag="cat1024")
            for cb in range(CB):
                for nh in range(2):
                    pff = ps_acc.tile([P, 512], FP, tag=f"acc{nh}", bufs=1)
                    for kb in range(NB):
                        _mm(nc, pff,
                            wn2[:, kb, cb * P:(cb + 1) * P],
                            h1T[:, kb, nh * 512:(nh + 1) * 512],
                            start=(kb == 0), stop=(kb == NB - 1))
                    nc.vector.tensor_scalar_add(
                        out=ffT[:, cb, nh * 512:(nh + 1) * 512],
                        in0=pff,
                        scalar1=bn2[:, cb:cb + 1])

            # residual + LN2; rebuild nfT (XT[0:2]) and nf16
            g2nB = load_bcast(w["g2n"], C, "gB")
            b2nB = load_bcast(w["b2n"], C, "bB2")
            for nb in range(NB):
                for cb in range(CB):
                    pt = ps_tr.tile([P, P], FP, tag="tr")
                    nc.tensor.transpose(pt, ffT[:, cb, nb * P:(nb + 1) * P],
                                        ident)
                    nc.vector.tensor_add(
                        out=nf_tok[:, nb, cb * P:(cb + 1) * P],
                        in0=nf_tok[:, nb, cb * P:(cb + 1) * P], in1=pt)
                _ln_tok(nc, small, nf_tok[:, nb, :], g2nB, b2nB, eps_t)
                nc.vector.tensor_copy(out=nf16[:, nb, :], in_=nf_tok[:, nb, :])
            for nb in range(NB):
                for cb in range(CB):
                    transpose_128(nf_tok[:, nb, cb * P:(cb + 1) * P],
                                  XT[:, cb, nb * P:(nb + 1) * P])

            # ---------------- edge phase ----------------
            we1a = load_w(w["we1a"], 2 * C, HID, "w_a")
            be1a = load_b(w["be1a"], HID, "b_hid")
            we1b = load_w(w["we1b"], HID, C, "w_b")
            be1b = load_b(w["be1b"], C, "b_a")
            we2a = load_w(w["we2a"], C, HID, "w_c")
            be2a = load_b(w["be2a"], HID, "b_hid2")
            we2b = load_w(w["we2b"], HID, C, "w_d")
            be2b = load_b(w["be2b"], C, "b_b")
            g1eB = load_bcast(w["g1e"], C, "gB")
            b1eB = load_bcast(w["b1e"], C, "bB2")
            g2eB = load_bcast(w["g2e"], C, "gB2")
            b2eB = load_bcast(w["b2e"], C, "bB3")

            for ec in range(NCH):
                esl = slice(ec * ECH, (ec + 1) * ECH)
                # eaggT [2cb, 256e] = nf^T(G/div_e)^T  (feature-major)
                gtt = scr.tile([P, NB, ECH], MD, tag="gts_t", bufs=1)
                nc.sync.dma_start(gtt, gts_r[:, :, esl])
                eaggT = scr.tile([P, CB, ECH], MD, tag="eaggT", bufs=2)
                for cb in range(CB):
                    pt = ps.tile([P, ECH], FP, tag="mm")
                    for kb in range(NB):
                        _mm(nc, pt, nf16[:, kb, cb * P:(cb + 1) * P],
                            gtt[:, kb, :], start=(kb == 0),
                            stop=(kb == NB - 1))
                    nc.scalar.activation(out=eaggT[:, cb, :], in_=pt,
                         func=mybir.ActivationFunctionType.Copy)
                # efT chunk (feature-major, pre-LN ef)
                efTc = scr.tile([P, CB, ECH], MD, tag="efTc", bufs=2)
                for eb in range(ECH // P):
                    for cb in range(CB):
                        transpose_128(
                            ef_tok[:, ec * (ECH // P) + eb, cb * P:(cb + 1) * P],
                            efTc[:, cb, eb * P:(eb + 1) * P])
                # h1 = relu([ef, eagg] @ We1a + b)
                h1e = scr.tile([P, HID // P, ECH], MD, tag="h1e", bufs=2)
                for hb in range(HID // P):
                    pt = ps.tile([P, ECH], FP, tag="mm")
                    for kb in range(CB):
                        _mm(nc, pt, we1a[:, kb, hb * P:(hb + 1) * P],
                            efTc[:, kb, :], start=(kb == 0), stop=False)
                    for kb in range(CB):
                        _mm(nc, pt, we1a[:, CB + kb, hb * P:(hb + 1) * P],
                            eaggT[:, kb, :], start=False, stop=(kb == CB - 1))
                    nc.vector.tensor_scalar(
                        out=h1e[:, hb, :], in0=pt,
                        scalar1=be1a[:, hb:hb + 1], scalar2=0.0,
                        op0=mybir.AluOpType.add, op1=mybir.AluOpType.max)
                # e1T = We1b^T @ h1 + b   (fp32, feeds transpose/residual)
                e1T = scr.tile([P, CB, ECH], FP, tag="e1T", bufs=2)
                for cb in range(CB):
                    pt = ps.tile([P, ECH], FP, tag="mm")
                    for kb in range(HID // P):
                        _mm(nc, pt, we1b[:, kb, cb * P:(cb + 1) * P],
                            h1e[:, kb, :], start=(kb == 0),
                            stop=(kb == HID // P - 1))
                    nc.vector.tensor_scalar_add(
                        out=e1T[:, cb, :], in0=pt,
                        scalar1=be1b[:, cb:cb + 1])
                # residual + LN1 (token-major)
                for eb in range(ECH // P):
                    for cb in range(CB):
                        pt = ps_tr.tile([P, P], FP, tag="tr")
                        nc.tensor.transpose(pt,
                                            e1T[:, cb, eb * P:(eb + 1) * P],
                                            ident)
                        nc.vector.tensor_add(
                            out=ef_tok[:, ec * (ECH // P) + eb,
                                       cb * P:(cb + 1) * P],
                            in0=ef_tok[:, ec * (ECH // P) + eb,
                                       cb * P:(cb + 1) * P],
                            in1=pt)
                    _ln_tok(nc, small, ef_tok[:, ec * (ECH // P) + eb, :],
                            g1eB, b1eB, eps_t)
                # efT of LN1 output
                for eb in range(ECH // P):
                    for cb in range(CB):
                        transpose_128(
                            ef_tok[:, ec * (ECH // P) + eb, cb * P:(cb + 1) * P],
                            efTc[:, cb, eb * P:(eb + 1) * P])
                # h2 = relu(ef @ We2a + b)
                for hb in range(HID // P):
                    pt = ps.tile([P, ECH], FP, tag="mm")
                    for kb in range(CB):
                        _mm(nc, pt, we2a[:, kb, hb * P:(hb + 1) * P],
                            efTc[:, kb, :], start=(kb == 0),
                            stop=(kb == CB - 1))
                    nc.vector.tensor_scalar(
                        out=h1e[:, hb, :], in0=pt,
                        scalar1=be2a[:, hb:hb + 1], scalar2=0.0,
                        op0=mybir.AluOpType.add, op1=mybir.AluOpType.max)
                # e2T = We2b^T @ h2 + b
                for cb in range(CB):
                    pt = ps.tile([P, ECH], FP, tag="mm")
                    for kb in range(HID // P):
                        _mm(nc, pt, we2b[:, kb, cb * P:(cb + 1) * P],
                            h1e[:, kb, :], start=(kb == 0),
                            stop=(kb == HID // P - 1))
                    nc.vector.tensor_scalar_add(
                        out=e1T[:, cb, :], in0=pt,
                        scalar1=be2b[:, cb:cb + 1])
                # residual + LN2; refresh ef16
                for eb in range(ECH // P):
                    for cb in range(CB):
                        pt = ps_tr.tile([P, P], FP, tag="tr")
                        nc.tensor.transpose(pt,
                                            e1T[:, cb, eb * P:(eb + 1) * P],
                                            ident)
                        nc.vector.tensor_add(
                            out=ef_tok[:, ec * (ECH // P) + eb,
                                       cb * P:(cb + 1) * P],
                            in0=ef_tok[:, ec * (ECH // P) + eb,
                                       cb * P:(cb + 1) * P],
                            in1=pt)
                    _ln_tok(nc, small, ef_tok[:, ec * (ECH // P) + eb, :],
                            g2eB, b2eB, eps_t)
                    if li < NL - 1:
                        nc.vector.tensor_copy(
                            out=ef16[:, ec * (ECH // P) + eb, :],
                            in_=ef_tok[:, ec * (ECH // P) + eb, :])

        # output
        nc.sync.dma_start(nf_out[:].rearrange("(o p) c -> p o c", p=P), nf_tok)

    nc.compile()
    return nc


def _prep_host(node_features, edge_features, G, params):
    nf = np.ascontiguousarray(np.asarray(node_features, np.float32))
    ef = np.ascontiguousarray(np.asarray(edge_features, np.float32))
    Gf = np.asarray(G, np.float32)
    div = Gf.sum(axis=1)                      # [B, N]
    div_e = Gf.sum(axis=2)                    # [B, E]
    gdiv = np.ascontiguousarray((Gf / div[:, None, :]).astype(MD_NP))
    gts = np.ascontiguousarray(
        (Gf / div_e[:, :, None]).transpose(0, 2, 1).astype(MD_NP))
    nfT = np.ascontiguousarray(nf.transpose(0, 2, 1).astype(MD_NP))
    ef16 = ef.astype(MD_NP)

    ind = np.zeros((H, C), np.float32)
    for h in range(H):
        ind[h, HD * h:HD * (h + 1)] = 1.0
    ind = ind.astype(MD_NP)

    layers = []
    for p in params:
        p = {k: np.asarray(v, np.float32) for k, v in p.items()}
        Wqn = p["Wqn"].reshape(C, H, 3, HD)
        Wqe = p["Wqe"].reshape(C, H, 3, HD)
        bqn = p["bqn"].reshape(H, 3, HD)
        bqe = p["bqe"].reshape(H, 3, HD)
        bsum = bqn + bqe
        wq = np.concatenate([Wqn[:, :, 0].reshape(C, C),
                             Wqe[:, :, 0].reshape(C, C)], axis=0)
        wk = np.concatenate([Wqn[:, :, 1].reshape(C, C),
                             Wqe[:, :, 1].reshape(C, C)], axis=0)
        wv = np.concatenate([Wqn[:, :, 2].reshape(C, C),
                             Wqe[:, :, 2].reshape(C, C)], axis=0)
        d = {
            "wqk": np.concatenate([wq, wk], axis=1).astype(MD_NP),
            "bqk": np.concatenate([bsum[:, 0].reshape(C),
                                   bsum[:, 1].reshape(C)]),
            "wv": wv.astype(MD_NP),
            "bv": bsum[:, 2].reshape(C),
            "wo": p["Wo"].astype(MD_NP), "bo": p["bo"],
            "wn1": p["Wn1"].astype(MD_NP), "bn1": p["bn1"],
            "wn2": p["Wn2"].astype(MD_NP), "bn2": p["bn2"],
            "we1a": p["We1a"].astype(MD_NP), "be1a": p["be1a"],
            "we1b": p["We1b"].astype(MD_NP), "be1b": p["be1b"],
            "we2a": p["We2a"].astype(MD_NP), "be2a": p["be2a"],
            "we2b": p["We2b"].astype(MD_NP), "be2b": p["be2b"],
            "g1n": p["g1n"], "b1n": p["b1n"], "g2n": p["g2n"], "b2n": p["b2n"],
            "g1e": p["g1e"], "b1e": p["b1e"], "g2e": p["g2e"], "b2e": p["b2e"],
        }
        layers.append({k: np.ascontiguousarray(v) for k, v in d.items()})
    return nf, nfT, ef, ef16, gdiv, gts, ind, layers


_NC_CACHE = None


def kernel(node_features, edge_features, G, params):
    global LAST_RESULTS, _NC_CACHE
    nf, nfT, ef, ef16, gdiv, gts, ind, layers = _prep_host(
        node_features, edge_features, G, params)

    if _NC_CACHE is None:
        _NC_CACHE = build_kernel()
    nc = _NC_CACHE

    shared = {"ind": ind}
    for i, d in enumerate(layers):
        for k, v in d.items():
            shared[f"L{i}_{k}"] = v
    in_maps = []
    for b in range(B):
        m = dict(shared)
        m["nf_in"] = nf[b]
        m["nfT_in"] = nfT[b]
        m["ef_in"] = ef[b]
        m["ef16_in"] = ef16[b]
        m["gdiv"] = gdiv[b]
        m["gts"] = gts[b]
        in_maps.append(m)

    res = run_bass_kernel_spmd(nc, in_maps, core_ids=list(range(B)))
    LAST_RESULTS = res
    out = np.stack([res.results[b]["nf_out"] for b in range(B)], axis=0)
    return (out, np.asarray(G))
